# revision 1
# baseline (speedup 1.0000x reference)
"""Causal GQA self-attention with RoPE on 8 Trainium2 NeuronCores.

Sharding: tensor-parallel over heads. Each core owns 2 q-heads and their
(shared) kv-head: it projects q/k/v for all 4096 tokens, applies RoPE, runs
causal attention, then an AllToAll redistributes attention outputs so each
core o-projects a 512-token slice with the full Wo. Host assembles slices.

All matmuls run in bf16 with fp32 PSUM accumulation. The RoPE interleaved
pair rotation is turned into a contiguous half rotation by permuting the
rows of Wq/Wk on the host (even head-dims first); q.k dot products are
invariant under applying the same permutation to q and k.

Shapes (hardcoded from the problem spec):
  x [2, 2048, 2048] f32, Wq [2048, 2048], Wk/Wv [512, 2048], Wo [2048, 2048],
  position_ids [2048] i32.  16 q-heads, 4 kv-heads, head_dim 128.
"""

from contextlib import ExitStack

import ml_dtypes
import numpy as np

import concourse.bass as bass
import concourse.tile as tile
from concourse import bacc, mybir
from concourse.bass_utils import run_bass_kernel_spmd
from concourse.masks import make_identity

B, S, E = 2, 2048, 2048
H, HKV, D = 16, 4, 128
NCORES = 8
HPC = H // NCORES          # q-heads per core
T = B * S                  # 4096 flattened tokens
TSL = T // NCORES          # 512-token o_proj slice per core
NE = E // 128              # 16 contraction chunks
ROPE_THETA = 10000.0
SCALE = 1.0 / float(np.sqrt(D))

BF16 = mybir.dt.bfloat16
FP32 = mybir.dt.float32
AF = mybir.ActivationFunctionType

_cached_nc = None


def _build_nc(single=False, repeats=1):
    """single=True: 1-core variant with the collective replaced by a DMA copy
    (for TimelineSim cost-model analysis only).  repeats>1 unrolls the whole
    body N times inside one NEFF (used to measure steady-state device time
    by slope, since per-dispatch host overhead dominates wall clock)."""
    nc = bacc.Bacc(None, target_bir_lowering=False, debug=False,
                   num_devices=1 if single else NCORES)

    xT = nc.dram_tensor("xT", [E, T], BF16, kind="ExternalInput")
    wqkvT = nc.dram_tensor("wqkvT", [E, 512], BF16, kind="ExternalInput")
    woT = nc.dram_tensor("woT", [E, E], BF16, kind="ExternalInput")
    cosT = nc.dram_tensor("cosT", [128, T], BF16, kind="ExternalInput")
    sinT = nc.dram_tensor("sinT", [128, T], BF16, kind="ExternalInput")
    masks = nc.dram_tensor("masks", [128, 896], BF16, kind="ExternalInput")
    outT = nc.dram_tensor("outT", [E, TSL], FP32, kind="ExternalOutput")

    a2a_in = nc.dram_tensor("a2a_in", [NCORES * HPC * D, TSL], BF16)
    a2a_out = nc.dram_tensor("a2a_out", [NCORES * HPC * D, TSL], BF16)

    def pair(dram, r0, c0, c1):
        # [256 rows, c] dram block -> [128, 2, c] AP (two row-chunks side
        # by side) so two contraction chunks load in one DMA
        return dram[r0:r0 + 256, c0:c1].rearrange("(c p) t -> p c t", c=2)

    def as3(ap, c=2):
        return ap.rearrange("p (c t) -> p c t", c=c)

    with tile.TileContext(nc) as tc, ExitStack() as ctx:
        # ---- persistent SBUF ----
        const_pool = ctx.enter_context(tc.tile_pool(name="const", bufs=1))
        qkv_pool = ctx.enter_context(tc.tile_pool(name="qkv", bufs=1))

        cos_sb = const_pool.tile([128, T], BF16, tag="cos")
        sin_sb = const_pool.tile([128, T], BF16, tag="sin")
        # causal band: mask_sb[k, j] = (k + 384 <= j); the r-th diagonal
        # 128-block mask is the slice [:, 384-128r : 896-128r]
        mask_sb = const_pool.tile([128, 896], BF16, tag="mask")

        ones_col = const_pool.tile([128, 1], BF16, tag="ones_col")
        nc.gpsimd.memset(ones_col[:], 1.0)

        # per (head, batch) q tiles and per-batch k/v tiles so attention on
        # batch 0 can start while batch 1 is still projecting.
        # v is stored transposed in 128-token column blocks:
        # v_sb[b][:, c*128:(c+1)*128] = v[tokens c*128...][:, d]
        q_sb = [[qkv_pool.tile([128, S], BF16, tag=f"q{h}{b}", name=f"q{h}{b}")
                 for b in range(B)] for h in range(HPC)]
        k_sb = [qkv_pool.tile([128, S], BF16, tag=f"k{b}", name=f"k{b}")
                for b in range(B)]
        v_sb = [qkv_pool.tile([128, S], BF16, tag=f"v{b}", name=f"v{b}")
                for b in range(B)]

        w_pool = ctx.enter_context(tc.tile_pool(name="w", bufs=8))
        w_sb = [w_pool.tile([128, 1024], BF16, tag="wqkv", name=f"w{e}")
                for e in range(8)]

        for _rep in range(repeats):
            # ---- phase 1: qkv projection + rope + v transpose ----
            phase1 = ExitStack()
            x_pool = phase1.enter_context(tc.tile_pool(name="x", bufs=10))
            proj_psum = phase1.enter_context(tc.tile_pool(name="pproj", bufs=4, space="PSUM"))
            tr_psum = phase1.enter_context(tc.tile_pool(name="ptr", bufs=2, space="PSUM"))
            rope_tmp = phase1.enter_context(tc.tile_pool(name="ropetmp", bufs=8))
            v_tmp = phase1.enter_context(tc.tile_pool(name="vtmp", bufs=2))

            QT = 1024  # tokens per quarter

            # first-quarter x and w loads go first so the first matmuls
            # aren't stuck behind the (cold) table/mask loads
            xq0 = []
            for e2 in range(8):
                xt = x_pool.tile([128, 2 * QT], BF16, tag="x", name=f"x0_{e2}")
                nc.sync.dma_start(as3(xt[:]), pair(xT, e2 * 256, 0, QT))
                xq0.append(xt)
                if _rep == 0:
                    nc.scalar.dma_start(as3(w_sb[e2][:]),
                                        pair(wqkvT, e2 * 256, 0, 512))
            if _rep == 0:
                nc.scalar.dma_start(cos_sb[:], cosT[:, :])
                nc.scalar.dma_start(sin_sb[:], sinT[:, :])
                nc.scalar.dma_start(mask_sb[:], masks[:, :])

            for tq in range(T // QT):
                b = tq // 2            # batch this quarter belongs to
                if tq == 0:
                    xq = xq0
                else:
                    xq = []
                    for e2 in range(8):
                        xt = x_pool.tile([128, 2 * QT], BF16, tag="x")
                        nc.sync.dma_start(
                            as3(xt[:]), pair(xT, e2 * 256, tq * QT, (tq + 1) * QT))
                        xq.append(xt)
                # v projected directly in [token, dim] orientation (x tile is
                # the stationary operand) so no PE transpose is needed
                for tb in range(QT // 128):
                    pv = tr_psum.tile([128, 128], FP32, tag="tr")
                    for e in range(NE):
                        nc.tensor.matmul(
                            pv[:],
                            lhsT=xq[e // 2][:, (e % 2) * QT + tb * 128:
                                            (e % 2) * QT + (tb + 1) * 128],
                            rhs=w_sb[e // 2][:, (e % 2) * 512 + 3 * 128:
                                             (e % 2) * 512 + 4 * 128],
                            start=(e == 0), stop=(e == NE - 1),
                        )
                    cblk = tq * QT + tb * 128 - b * S
                    nc.vector.tensor_copy(v_sb[b][:, cblk:cblk + 128], pv[:])
                for dt in range(3):  # q-head0, q-head1, k
                    for half in range(2):
                        ps = proj_psum.tile([128, 512], FP32, tag="proj")
                        for e in range(NE):
                            nc.tensor.matmul(
                                ps[:],
                                lhsT=w_sb[e // 2][:, (e % 2) * 512 + dt * 128:
                                                  (e % 2) * 512 + (dt + 1) * 128],
                                rhs=xq[e // 2][:, (e % 2) * QT + half * 512:
                                               (e % 2) * QT + (half + 1) * 512],
                                start=(e == 0),
                                stop=(e == NE - 1),
                            )
                        gcol = tq * QT + half * 512   # global token offset
                        col = gcol - b * S            # within-batch offset
                        # evacuate psum once (cast to bf16); DVE rope work
                        # stays in fast bf16-SBUF mode
                        vt = v_tmp.tile([128, 512], BF16, tag="vt")
                        nc.scalar.copy(vt[:], ps[:])
                        if True:
                            # rope: rows 0:64 = even head dims, 64:128 = odd
                            dst = q_sb[dt][b] if dt < 2 else k_sb[b]
                            c_top = cos_sb[0:64, gcol:gcol + 512]
                            c_bot = cos_sb[64:128, gcol:gcol + 512]
                            s_top = sin_sb[0:64, gcol:gcol + 512]
                            s_bot = sin_sb[64:128, gcol:gcol + 512]
                            t1 = rope_tmp.tile([64, 512], BF16, tag="rt")
                            t2 = rope_tmp.tile([64, 512], BF16, tag="rt")
                            t3 = rope_tmp.tile([64, 512], BF16, tag="rt")
                            t4 = rope_tmp.tile([64, 512], BF16, tag="rt")
                            top, bot = vt[0:64, :], vt[64:128, :]
                            nc.vector.tensor_mul(t1[:], top, c_top)
                            nc.vector.tensor_mul(t2[:], bot, s_bot)
                            nc.vector.tensor_sub(dst[0:64, col:col + 512], t1[:], t2[:])
                            nc.vector.tensor_mul(t3[:], bot, c_bot)
                            nc.vector.tensor_mul(t4[:], top, s_top)
                            nc.vector.tensor_add(dst[64:128, col:col + 512], t3[:], t4[:])

            phase1.close()

            # ---- phase 2: causal attention per (batch, head) ----
            if _rep == 0:
                wo_pool = ctx.enter_context(tc.tile_pool(name="wo", bufs=10))
            phase2 = ExitStack()
            s_psum = phase2.enter_context(tc.tile_pool(name="ps", bufs=4, space="PSUM"))
            y_psum = phase2.enter_context(tc.tile_pool(name="py", bufs=2, space="PSUM"))
            d_psum = phase2.enter_context(tc.tile_pool(name="pd", bufs=2, space="PSUM"))
            e_pool = phase2.enter_context(tc.tile_pool(name="e", bufs=8))
            sm_pool = phase2.enter_context(tc.tile_pool(name="sm", bufs=2))

            for b in range(B):
                for h in range(HPC):
                    for qc in range(S // 512):
                        q0 = qc * 512
                        nkt = (q0 + 512) // 128
                        ps_y = y_psum.tile([128, 512], FP32, tag="y")
                        ps_d = d_psum.tile([1, 512], FP32, tag="d")
                        for kt in range(nkt):
                            ps_s = s_psum.tile([128, 512], FP32, tag="s")
                            nc.tensor.matmul(
                                ps_s[:],
                                lhsT=k_sb[b][:, kt * 128:(kt + 1) * 128],
                                rhs=q_sb[h][b][:, q0:q0 + 512],
                                start=True, stop=True,
                            )
                            e_t = e_pool.tile([128, 512], BF16, tag="e")
                            nc.scalar.activation(e_t[:], ps_s[:], AF.Exp, scale=SCALE)
                            r = kt * 128 - q0
                            if r >= 0:
                                off = 384 - r
                                nc.vector.tensor_mul(
                                    e_t[:], e_t[:], mask_sb[:, off:off + 512])
                            nc.tensor.matmul(
                                ps_y[:],
                                lhsT=v_sb[b][:, kt * 128:(kt + 1) * 128],
                                rhs=e_t[:],
                                start=(kt == 0), stop=(kt == nkt - 1),
                            )
                            # softmax denominator accumulates on PE
                            nc.tensor.matmul(
                                ps_d[:], lhsT=ones_col[:], rhs=e_t[:],
                                start=(kt == 0), stop=(kt == nkt - 1),
                            )
                        # reciprocal -> partition broadcast (POOL) -> scale
                        recip = sm_pool.tile([1, 512], FP32, tag="recip")
                        nc.vector.reciprocal(recip[:], ps_d[:])
                        rb = sm_pool.tile([128, 512], FP32, tag="rb")
                        nc.gpsimd.partition_broadcast(rb[:], recip[:])
                        y_t = sm_pool.tile([128, 512], BF16, tag="yt")
                        nc.vector.tensor_mul(y_t[:], ps_y[:], rb[:])
                        # a2a_in shard j=(4b+qc): rows j*256 + h*128
                        j = 4 * b + qc
                        row = j * HPC * D + h * 128
                        nc.sync.dma_start(a2a_in[row:row + 128, :], y_t[:])

            # prefetch o_proj weights while attention still runs
            wo_t = []
            for g2 in range(2):
                for yd2 in range(8):
                    wt = wo_pool.tile([128, 2048], BF16, tag="wo",
                                      name=f"wo{g2}_{yd2}")
                    nc.scalar.dma_start(
                        as3(wt[:]),
                        pair(woT, yd2 * 256, g2 * 1024, (g2 + 1) * 1024))
                    wo_t.append(wt)

            # ---- all-to-all: heads -> token slices ----
            if single:
                nc.sync.dma_start(a2a_out[:, :], a2a_in[:, :])
            else:
                nc.gpsimd.collective_compute(
                    "AllToAll",
                    mybir.AluOpType.bypass,
                    replica_groups=[list(range(NCORES))],
                    ins=[a2a_in[:, :]],
                    outs=[a2a_out[:, :]],
                )

            phase2.close()

            # ---- phase 3: o_proj for this core's 512-token slice ----
            phase3 = ExitStack()
            yag_pool = phase3.enter_context(tc.tile_pool(name="yag", bufs=8))
            o_psum = phase3.enter_context(tc.tile_pool(name="po", bufs=2, space="PSUM"))
            o_pool = phase3.enter_context(tc.tile_pool(name="osb", bufs=2))

            yag = []

            for yd2 in range(8):

                yt = yag_pool.tile([128, 2 * TSL], BF16, tag="yag")

                nc.sync.dma_start(as3(yt[:]), pair(a2a_out, yd2 * 256, 0, TSL))

                yag.append(yt)
            for ot in range(NE):
                ps_o = o_psum.tile([128, TSL], FP32, tag="o")
                for yd in range(NE):
                    wt = wo_t[(ot // 8) * 8 + yd // 2]
                    oi = ot % 8
                    nc.tensor.matmul(
                        ps_o[:],
                        lhsT=wt[:, (yd % 2) * 1024 + oi * 128:
                                (yd % 2) * 1024 + (oi + 1) * 128],
                        rhs=yag[yd // 2][:, (yd % 2) * TSL:(yd % 2 + 1) * TSL],
                        start=(yd == 0), stop=(yd == NE - 1))
                o_sb = o_pool.tile([128, TSL], FP32, tag="osb")
                nc.vector.tensor_copy(o_sb[:], ps_o[:])
                nc.sync.dma_start(outT[ot * 128:(ot + 1) * 128, :], o_sb[:])
            phase3.close()

    nc.compile()
    return nc


def _prep_inputs(x, Wq, Wk, Wv, Wo, position_ids):
    bf16 = ml_dtypes.bfloat16
    xT = np.ascontiguousarray(
        x.reshape(T, E).T).astype(bf16)

    # rope permutation: even head-dims first, then odd
    perm = np.concatenate([np.arange(0, D, 2), np.arange(1, D, 2)])
    Wq_p = Wq.reshape(H, D, E)[:, perm, :]
    Wk_p = Wk.reshape(HKV, D, E)[:, perm, :]
    Wv_r = Wv.reshape(HKV, D, E)

    pos = position_ids.astype(np.float64)
    inv_freq = 1.0 / (ROPE_THETA ** (np.arange(0, D, 2, dtype=np.float64) / D))
    freqs = pos[:, None] * inv_freq[None, :]            # [S, 64]
    cosT = np.tile(np.cos(freqs).T, (2, B)).astype(bf16)  # [128, T] (dup halves)
    sinT = np.tile(np.sin(freqs).T, (2, B)).astype(bf16)

    # causal band mask: masks[k, j] = 1 if k + 384 <= j; the r-th diagonal
    # block mask [k, q] = (k + 128r <= q) is the slice [:, 384-128r:896-128r]
    kk = np.arange(128)[:, None]
    jj = np.arange(896)[None, :]
    masks = np.ascontiguousarray(
        (kk + 384 <= jj).astype(np.float32)).astype(bf16)

    woT = np.ascontiguousarray(Wo.T).astype(bf16)       # [yd, o]

    in_maps = []
    for c in range(NCORES):
        wq_c = Wq_p[2 * c:2 * c + 2].reshape(HPC * D, E)     # [256, E]
        g = c // 2
        wqkvT = np.concatenate(
            [wq_c.T, Wk_p[g].T, Wv_r[g].T], axis=1).astype(bf16)  # [E, 512]
        in_maps.append({
            "xT": xT,
            "wqkvT": np.ascontiguousarray(wqkvT),
            "woT": woT,
            "cosT": cosT,
            "sinT": sinT,
            "masks": masks,
        })
    return in_maps


def kernel(x, Wq, Wk, Wv, Wo, position_ids):
    global _cached_nc
    if _cached_nc is None:
        _cached_nc = _build_nc()
    nc = _cached_nc

    in_maps = _prep_inputs(
        np.asarray(x, np.float32), np.asarray(Wq, np.float32),
        np.asarray(Wk, np.float32), np.asarray(Wv, np.float32),
        np.asarray(Wo, np.float32), np.asarray(position_ids))

    res = run_bass_kernel_spmd(nc, in_maps, core_ids=list(range(NCORES)))

    out = np.concatenate(
        [res.results[c]["outT"].T for c in range(NCORES)], axis=0)
    return np.ascontiguousarray(out.reshape(B, S, E).astype(np.float32))



# revision 6
# speedup vs baseline: 1.0377x; 1.0377x over previous
"""Causal GQA self-attention with RoPE on 8 Trainium2 NeuronCores.

Sharding: tensor-parallel over heads. Each core owns 2 q-heads and their
(shared) kv-head: it projects q/k/v for all 4096 tokens, applies RoPE, runs
causal attention, then an AllToAll redistributes attention outputs so each
core o-projects a 512-token slice with the full Wo. Host assembles slices.

All matmuls run in bf16 with fp32 PSUM accumulation. The RoPE interleaved
pair rotation is turned into a contiguous half rotation by permuting the
rows of Wq/Wk on the host (even head-dims first); q.k dot products are
invariant under applying the same permutation to q and k.

Shapes (hardcoded from the problem spec):
  x [2, 2048, 2048] f32, Wq [2048, 2048], Wk/Wv [512, 2048], Wo [2048, 2048],
  position_ids [2048] i32.  16 q-heads, 4 kv-heads, head_dim 128.
"""

from contextlib import ExitStack

import ml_dtypes
import numpy as np

import concourse.bass as bass
import concourse.tile as tile
from concourse import bacc, mybir
from concourse.bass_utils import run_bass_kernel_spmd
from concourse.masks import make_identity

B, S, E = 2, 2048, 2048
H, HKV, D = 16, 4, 128
NCORES = 8
HPC = H // NCORES          # q-heads per core
T = B * S                  # 4096 flattened tokens
TSL = T // NCORES          # 512-token o_proj slice per core
NE = E // 128              # 16 contraction chunks
ROPE_THETA = 10000.0
SCALE = 1.0 / float(np.sqrt(D))

BF16 = mybir.dt.bfloat16
FP32 = mybir.dt.float32
AF = mybir.ActivationFunctionType

_cached_nc = None


def _build_nc(single=False, repeats=1):
    """single=True: 1-core variant with the collective replaced by a DMA copy
    (for TimelineSim cost-model analysis only).  repeats>1 unrolls the whole
    body N times inside one NEFF (used to measure steady-state device time
    by slope, since per-dispatch host overhead dominates wall clock)."""
    nc = bacc.Bacc(None, target_bir_lowering=False, debug=False,
                   num_devices=1 if single else NCORES)

    xT = nc.dram_tensor("xT", [E, T], BF16, kind="ExternalInput")
    wqkvT = nc.dram_tensor("wqkvT", [E, 512], BF16, kind="ExternalInput")
    woT = nc.dram_tensor("woT", [E, E], BF16, kind="ExternalInput")
    cosT = nc.dram_tensor("cosT", [128, T], BF16, kind="ExternalInput")
    sinT = nc.dram_tensor("sinT", [128, T], BF16, kind="ExternalInput")
    masks = nc.dram_tensor("masks", [128, 896], BF16, kind="ExternalInput")
    outT = nc.dram_tensor("outT", [E, TSL], FP32, kind="ExternalOutput")

    # per-batch all-to-all buffers: dest core j gets tokens [j*256,(j+1)*256)
    # of that batch, so each collective is half-size and overlaps compute
    HSL = TSL // 2   # 256 tokens per (core, batch) output slice
    a2a_in = [nc.dram_tensor(f"a2a_in{b}", [NCORES * HPC * D, HSL], BF16)
              for b in range(B)]
    a2a_out = [nc.dram_tensor(f"a2a_out{b}", [NCORES * HPC * D, HSL], BF16)
               for b in range(B)]

    def pair(dram, r0, c0, c1):
        # [256 rows, c] dram block -> [128, 2, c] AP (two row-chunks side
        # by side) so two contraction chunks load in one DMA
        return dram[r0:r0 + 256, c0:c1].rearrange("(c p) t -> p c t", c=2)

    def as3(ap, c=2):
        return ap.rearrange("p (c t) -> p c t", c=c)

    with tile.TileContext(nc) as tc, ExitStack() as ctx:
        # ---- persistent SBUF ----
        const_pool = ctx.enter_context(tc.tile_pool(name="const", bufs=1))
        qkv_pool = ctx.enter_context(tc.tile_pool(name="qkv", bufs=1))

        cos_sb = const_pool.tile([128, T], BF16, tag="cos")
        sin_sb = const_pool.tile([128, T], BF16, tag="sin")
        # causal band: mask_sb[k, j] = (k + 384 <= j); the r-th diagonal
        # 128-block mask is the slice [:, 384-128r : 896-128r]
        mask_sb = const_pool.tile([128, 896], BF16, tag="mask")

        ones_col = const_pool.tile([128, 1], BF16, tag="ones_col")
        nc.gpsimd.memset(ones_col[:], 1.0)

        # per (head, batch) q tiles and per-batch k/v tiles so attention on
        # batch 0 can start while batch 1 is still projecting.
        # v is stored transposed in 128-token column blocks:
        # v_sb[b][:, c*128:(c+1)*128] = v[tokens c*128...][:, d]
        q_sb = [[qkv_pool.tile([128, S], BF16, tag=f"q{h}{b}", name=f"q{h}{b}")
                 for b in range(B)] for h in range(HPC)]
        k_sb = [qkv_pool.tile([128, S], BF16, tag=f"k{b}", name=f"k{b}")
                for b in range(B)]
        v_sb = [qkv_pool.tile([128, S], BF16, tag=f"v{b}", name=f"v{b}")
                for b in range(B)]

        w_pool = ctx.enter_context(tc.tile_pool(name="w", bufs=8))
        w_sb = [w_pool.tile([128, 1024], BF16, tag="wqkv", name=f"w{e}")
                for e in range(8)]

        for _rep in range(repeats):
            # ---- phase 1: qkv projection + rope + v transpose ----
            phase1 = ExitStack()
            x_pool = phase1.enter_context(tc.tile_pool(name="x", bufs=10))
            proj_psum = phase1.enter_context(tc.tile_pool(name="pproj", bufs=4, space="PSUM"))
            tr_psum = phase1.enter_context(tc.tile_pool(name="ptr", bufs=2, space="PSUM"))
            rope_tmp = phase1.enter_context(tc.tile_pool(name="ropetmp", bufs=8))
            v_tmp = phase1.enter_context(tc.tile_pool(name="vtmp", bufs=2))

            QT = 1024  # tokens per quarter

            # first-quarter x and w loads go first so the first matmuls
            # aren't stuck behind the (cold) table/mask loads
            xq0 = []
            for e2 in range(8):
                xt = x_pool.tile([128, 2 * QT], BF16, tag="x", name=f"x0_{e2}")
                nc.sync.dma_start(as3(xt[:]), pair(xT, e2 * 256, 0, QT))
                xq0.append(xt)
                if _rep == 0:
                    nc.scalar.dma_start(as3(w_sb[e2][:]),
                                        pair(wqkvT, e2 * 256, 0, 512))
            if _rep == 0:
                nc.scalar.dma_start(cos_sb[:], cosT[:, :])
                nc.scalar.dma_start(sin_sb[:], sinT[:, :])
                nc.scalar.dma_start(mask_sb[:], masks[:, :])

            for tq in range(T // QT):
                b = tq // 2            # batch this quarter belongs to
                if tq == 0:
                    xq = xq0
                else:
                    xq = []
                    for e2 in range(8):
                        xt = x_pool.tile([128, 2 * QT], BF16, tag="x")
                        nc.sync.dma_start(
                            as3(xt[:]), pair(xT, e2 * 256, tq * QT, (tq + 1) * QT))
                        xq.append(xt)
                # v projected directly in [token, dim] orientation (x tile is
                # the stationary operand) so no PE transpose is needed
                for tb in range(QT // 128):
                    pv = tr_psum.tile([128, 128], FP32, tag="tr")
                    for e in range(NE):
                        nc.tensor.matmul(
                            pv[:],
                            lhsT=xq[e // 2][:, (e % 2) * QT + tb * 128:
                                            (e % 2) * QT + (tb + 1) * 128],
                            rhs=w_sb[e // 2][:, (e % 2) * 512 + 3 * 128:
                                             (e % 2) * 512 + 4 * 128],
                            start=(e == 0), stop=(e == NE - 1),
                        )
                    cblk = tq * QT + tb * 128 - b * S
                    nc.vector.tensor_copy(v_sb[b][:, cblk:cblk + 128], pv[:])
                for dt in range(3):  # q-head0, q-head1, k
                    for half in range(2):
                        ps = proj_psum.tile([128, 512], FP32, tag="proj")
                        for e in range(NE):
                            nc.tensor.matmul(
                                ps[:],
                                lhsT=w_sb[e // 2][:, (e % 2) * 512 + dt * 128:
                                                  (e % 2) * 512 + (dt + 1) * 128],
                                rhs=xq[e // 2][:, (e % 2) * QT + half * 512:
                                               (e % 2) * QT + (half + 1) * 512],
                                start=(e == 0),
                                stop=(e == NE - 1),
                            )
                        gcol = tq * QT + half * 512   # global token offset
                        col = gcol - b * S            # within-batch offset
                        # evacuate psum once (cast to bf16); DVE rope work
                        # stays in fast bf16-SBUF mode
                        vt = v_tmp.tile([128, 512], BF16, tag="vt")
                        nc.scalar.copy(vt[:], ps[:])
                        if True:
                            # rope: rows 0:64 = even head dims, 64:128 = odd
                            dst = q_sb[dt][b] if dt < 2 else k_sb[b]
                            c_top = cos_sb[0:64, gcol:gcol + 512]
                            c_bot = cos_sb[64:128, gcol:gcol + 512]
                            s_top = sin_sb[0:64, gcol:gcol + 512]
                            s_bot = sin_sb[64:128, gcol:gcol + 512]
                            t1 = rope_tmp.tile([64, 512], BF16, tag="rt")
                            t2 = rope_tmp.tile([64, 512], BF16, tag="rt")
                            t3 = rope_tmp.tile([64, 512], BF16, tag="rt")
                            t4 = rope_tmp.tile([64, 512], BF16, tag="rt")
                            top, bot = vt[0:64, :], vt[64:128, :]
                            nc.vector.tensor_mul(t1[:], top, c_top)
                            nc.vector.tensor_mul(t2[:], bot, s_bot)
                            nc.vector.tensor_sub(dst[0:64, col:col + 512], t1[:], t2[:])
                            nc.vector.tensor_mul(t3[:], bot, c_bot)
                            nc.vector.tensor_mul(t4[:], top, s_top)
                            nc.vector.tensor_add(dst[64:128, col:col + 512], t3[:], t4[:])

            phase1.close()

            # ---- phase 2: causal attention per (batch, head) ----
            if _rep == 0:
                wo_pool = ctx.enter_context(tc.tile_pool(name="wo", bufs=16))
            phase2 = ExitStack()
            s_psum = phase2.enter_context(tc.tile_pool(name="ps", bufs=4, space="PSUM"))
            y_psum = phase2.enter_context(tc.tile_pool(name="py", bufs=2, space="PSUM"))
            d_psum = phase2.enter_context(tc.tile_pool(name="pd", bufs=2, space="PSUM"))
            e_pool = phase2.enter_context(tc.tile_pool(name="e", bufs=10))
            d_pool = phase2.enter_context(tc.tile_pool(name="dq", bufs=6))
            sm_pool = phase2.enter_context(tc.tile_pool(name="sm", bufs=2))

            # prefetch o_proj weights under the whole attention phase
            wo_t = []
            for g2 in range(2):
                for yd2 in range(8):
                    wt = wo_pool.tile([128, 2048], BF16, tag="wo",
                                      name=f"wo{g2}_{yd2}")
                    nc.scalar.dma_start(
                        as3(wt[:]),
                        pair(woT, yd2 * 256, g2 * 1024, (g2 + 1) * 1024))
                    wo_t.append(wt)

            for b in range(B):
                for h in range(HPC):
                    for qc in range(S // 512):
                        q0 = qc * 512
                        nkt = (q0 + 512) // 128
                        ps_y = y_psum.tile([128, 512], FP32, tag="y")
                        ps_d = d_psum.tile([1, 512], FP32, tag="d")
                        for g in range(nkt // 4):
                            quad = []
                            for i4 in range(4):
                                kt = g * 4 + i4
                                ps_s = s_psum.tile([128, 512], FP32, tag="s")
                                nc.tensor.matmul(
                                    ps_s[:],
                                    lhsT=k_sb[b][:, kt * 128:(kt + 1) * 128],
                                    rhs=q_sb[h][b][:, q0:q0 + 512],
                                    start=True, stop=True,
                                )
                                e_t = e_pool.tile([128, 512], BF16, tag="e")
                                nc.scalar.activation(
                                    e_t[:], ps_s[:], AF.Exp, scale=SCALE)
                                r = kt * 128 - q0
                                if r >= 0:
                                    off = 384 - r
                                    nc.vector.tensor_mul(
                                        e_t[:], e_t[:], mask_sb[:, off:off + 512])
                                nc.tensor.matmul(
                                    ps_y[:],
                                    lhsT=v_sb[b][:, kt * 128:(kt + 1) * 128],
                                    rhs=e_t[:],
                                    start=(kt == 0), stop=(kt == nkt - 1),
                                )
                                quad.append(e_t)
                            # softmax denominator: tree-sum 4 e-tiles on DVE,
                            # then one PE matmul per quad (4x fewer than per-kt)
                            p0 = d_pool.tile([128, 512], BF16, tag="p")
                            p1 = d_pool.tile([128, 512], BF16, tag="p")
                            q_t = d_pool.tile([128, 512], BF16, tag="p")
                            nc.vector.tensor_add(p0[:], quad[0][:], quad[1][:])
                            nc.vector.tensor_add(p1[:], quad[2][:], quad[3][:])
                            nc.vector.tensor_add(q_t[:], p0[:], p1[:])
                            nc.tensor.matmul(
                                ps_d[:], lhsT=ones_col[:], rhs=q_t[:],
                                start=(g == 0), stop=(g == nkt // 4 - 1),
                            )
                        # reciprocal -> partition broadcast -> scale
                        recip = sm_pool.tile([1, 512], FP32, tag="recip")
                        nc.vector.reciprocal(recip[:], ps_d[:])
                        rb = sm_pool.tile([128, 512], FP32, tag="rb")
                        nc.gpsimd.partition_broadcast(rb[:], recip[:])
                        y_t = sm_pool.tile([128, 512], BF16, tag="yt")
                        nc.vector.tensor_mul(y_t[:], ps_y[:], rb[:])
                        # route the two 256-token halves to their dest cores
                        for hf in range(2):
                            j = 2 * qc + hf
                            row = j * HPC * D + h * 128
                            nc.sync.dma_start(
                                a2a_in[b][row:row + 128, :],
                                y_t[:, hf * HSL:(hf + 1) * HSL])
                # ---- per-batch all-to-all, overlapped with remaining work ----
                if single:
                    nc.sync.dma_start(a2a_out[b][:, :], a2a_in[b][:, :])
                else:
                    nc.gpsimd.collective_compute(
                        "AllToAll",
                        mybir.AluOpType.bypass,
                        replica_groups=[list(range(NCORES))],
                        ins=[a2a_in[b][:, :]],
                        outs=[a2a_out[b][:, :]],
                    )

            phase2.close()

            # ---- phase 3: o_proj, one 256-token half-slice per batch ----
            phase3 = ExitStack()
            yag_pool = phase3.enter_context(tc.tile_pool(name="yag", bufs=16))
            o_psum = phase3.enter_context(tc.tile_pool(name="po", bufs=4, space="PSUM"))
            o_pool = phase3.enter_context(tc.tile_pool(name="osb", bufs=4))

            for b in range(B):
                yag = []
                for yd2 in range(8):
                    yt = yag_pool.tile([128, 2 * HSL], BF16, tag="yag")
                    nc.sync.dma_start(
                        as3(yt[:]), pair(a2a_out[b], yd2 * 256, 0, HSL))
                    yag.append(yt)
                for ot in range(NE):
                    ps_o = o_psum.tile([128, HSL], FP32, tag="o")
                    for yd in range(NE):
                        wt = wo_t[(ot // 8) * 8 + yd // 2]
                        oi = ot % 8
                        nc.tensor.matmul(
                            ps_o[:],
                            lhsT=wt[:, (yd % 2) * 1024 + oi * 128:
                                    (yd % 2) * 1024 + (oi + 1) * 128],
                            rhs=yag[yd // 2][:, (yd % 2) * HSL:(yd % 2 + 1) * HSL],
                            start=(yd == 0), stop=(yd == NE - 1))
                    o_sb = o_pool.tile([128, HSL], FP32, tag="osb")
                    nc.vector.tensor_copy(o_sb[:], ps_o[:])
                    nc.sync.dma_start(
                        outT[ot * 128:(ot + 1) * 128, b * HSL:(b + 1) * HSL],
                        o_sb[:])
            phase3.close()

    nc.compile()
    return nc


def _prep_inputs(x, Wq, Wk, Wv, Wo, position_ids):
    bf16 = ml_dtypes.bfloat16
    xT = np.ascontiguousarray(
        x.reshape(T, E).T).astype(bf16)

    # rope permutation: even head-dims first, then odd
    perm = np.concatenate([np.arange(0, D, 2), np.arange(1, D, 2)])
    Wq_p = Wq.reshape(H, D, E)[:, perm, :]
    Wk_p = Wk.reshape(HKV, D, E)[:, perm, :]
    Wv_r = Wv.reshape(HKV, D, E)

    pos = position_ids.astype(np.float64)
    inv_freq = 1.0 / (ROPE_THETA ** (np.arange(0, D, 2, dtype=np.float64) / D))
    freqs = pos[:, None] * inv_freq[None, :]            # [S, 64]
    cosT = np.tile(np.cos(freqs).T, (2, B)).astype(bf16)  # [128, T] (dup halves)
    sinT = np.tile(np.sin(freqs).T, (2, B)).astype(bf16)

    # causal band mask: masks[k, j] = 1 if k + 384 <= j; the r-th diagonal
    # block mask [k, q] = (k + 128r <= q) is the slice [:, 384-128r:896-128r]
    kk = np.arange(128)[:, None]
    jj = np.arange(896)[None, :]
    masks = np.ascontiguousarray(
        (kk + 384 <= jj).astype(np.float32)).astype(bf16)

    woT = np.ascontiguousarray(Wo.T).astype(bf16)       # [yd, o]

    in_maps = []
    for c in range(NCORES):
        wq_c = Wq_p[2 * c:2 * c + 2].reshape(HPC * D, E)     # [256, E]
        g = c // 2
        wqkvT = np.concatenate(
            [wq_c.T, Wk_p[g].T, Wv_r[g].T], axis=1).astype(bf16)  # [E, 512]
        in_maps.append({
            "xT": xT,
            "wqkvT": np.ascontiguousarray(wqkvT),
            "woT": woT,
            "cosT": cosT,
            "sinT": sinT,
            "masks": masks,
        })
    return in_maps


def kernel(x, Wq, Wk, Wv, Wo, position_ids):
    global _cached_nc
    if _cached_nc is None:
        _cached_nc = _build_nc()
    nc = _cached_nc

    in_maps = _prep_inputs(
        np.asarray(x, np.float32), np.asarray(Wq, np.float32),
        np.asarray(Wk, np.float32), np.asarray(Wv, np.float32),
        np.asarray(Wo, np.float32), np.asarray(position_ids))

    res = run_bass_kernel_spmd(nc, in_maps, core_ids=list(range(NCORES)))

    # core c's outT is [E, 512]: cols 0:256 = batch-0 tokens [c*256,(c+1)*256),
    # cols 256:512 = batch-1 same token range
    HSL = TSL // 2
    out = np.empty((B, S, E), np.float32)
    for c in range(NCORES):
        o = res.results[c]["outT"]
        out[0, c * HSL:(c + 1) * HSL, :] = o[:, 0:HSL].T
        out[1, c * HSL:(c + 1) * HSL, :] = o[:, HSL:2 * HSL].T
    return np.ascontiguousarray(out)



# revision 7
# speedup vs baseline: 1.0776x; 1.0385x over previous
"""Causal GQA self-attention with RoPE on 8 Trainium2 NeuronCores.

Sharding: tensor-parallel over heads. Each core owns 2 q-heads and their
(shared) kv-head: it projects q/k/v for all 4096 tokens, applies RoPE, runs
causal attention, then an AllToAll redistributes attention outputs so each
core o-projects a 512-token slice with the full Wo. Host assembles slices.

All matmuls run in bf16 with fp32 PSUM accumulation. The RoPE interleaved
pair rotation is turned into a contiguous half rotation by permuting the
rows of Wq/Wk on the host (even head-dims first); q.k dot products are
invariant under applying the same permutation to q and k.

Shapes (hardcoded from the problem spec):
  x [2, 2048, 2048] f32, Wq [2048, 2048], Wk/Wv [512, 2048], Wo [2048, 2048],
  position_ids [2048] i32.  16 q-heads, 4 kv-heads, head_dim 128.
"""

from contextlib import ExitStack

import ml_dtypes
import numpy as np

import concourse.bass as bass
import concourse.tile as tile
from concourse import bacc, mybir
from concourse.bass_utils import run_bass_kernel_spmd
from concourse.masks import make_identity

B, S, E = 2, 2048, 2048
H, HKV, D = 16, 4, 128
NCORES = 8
HPC = H // NCORES          # q-heads per core
T = B * S                  # 4096 flattened tokens
TSL = T // NCORES          # 512-token o_proj slice per core
NE = E // 128              # 16 contraction chunks
ROPE_THETA = 10000.0
SCALE = 1.0 / float(np.sqrt(D))

BF16 = mybir.dt.bfloat16
FP32 = mybir.dt.float32
AF = mybir.ActivationFunctionType

_cached_nc = None


def _build_nc(single=False, repeats=1):
    """single=True: 1-core variant with the collective replaced by a DMA copy
    (for TimelineSim cost-model analysis only).  repeats>1 unrolls the whole
    body N times inside one NEFF (used to measure steady-state device time
    by slope, since per-dispatch host overhead dominates wall clock)."""
    nc = bacc.Bacc(None, target_bir_lowering=False, debug=False,
                   num_devices=1 if single else NCORES)

    xT = nc.dram_tensor("xT", [E, T], BF16, kind="ExternalInput")
    wqkvT = nc.dram_tensor("wqkvT", [E, 512], BF16, kind="ExternalInput")
    woT = nc.dram_tensor("woT", [E, E], BF16, kind="ExternalInput")
    cosT = nc.dram_tensor("cosT", [128, T], BF16, kind="ExternalInput")
    sinT = nc.dram_tensor("sinT", [128, T], BF16, kind="ExternalInput")
    masks = nc.dram_tensor("masks", [128, 896], BF16, kind="ExternalInput")
    outT = nc.dram_tensor("outT", [E, TSL], FP32, kind="ExternalOutput")

    # per-batch all-to-all buffers: dest core j gets tokens [j*256,(j+1)*256)
    # of that batch, so each collective is half-size and overlaps compute
    HSL = TSL // 2   # 256 tokens per (core, batch) output slice
    a2a_in = [nc.dram_tensor(f"a2a_in{b}", [NCORES * HPC * D, HSL], BF16)
              for b in range(B)]
    a2a_out = [nc.dram_tensor(f"a2a_out{b}", [NCORES * HPC * D, HSL], BF16)
               for b in range(B)]

    def pair(dram, r0, c0, c1):
        # [256 rows, c] dram block -> [128, 2, c] AP (two row-chunks side
        # by side) so two contraction chunks load in one DMA
        return dram[r0:r0 + 256, c0:c1].rearrange("(c p) t -> p c t", c=2)

    def as3(ap, c=2):
        return ap.rearrange("p (c t) -> p c t", c=c)

    with tile.TileContext(nc) as tc, ExitStack() as ctx:
        # ---- persistent SBUF ----
        const_pool = ctx.enter_context(tc.tile_pool(name="const", bufs=1))
        qkv_pool = ctx.enter_context(tc.tile_pool(name="qkv", bufs=1))

        cos_sb = const_pool.tile([128, T], BF16, tag="cos")
        sin_sb = const_pool.tile([128, T], BF16, tag="sin")
        # causal band: mask_sb[k, j] = (k + 384 <= j); the r-th diagonal
        # 128-block mask is the slice [:, 384-128r : 896-128r]
        mask_sb = const_pool.tile([128, 896], BF16, tag="mask")

        ones_col = const_pool.tile([128, 1], BF16, tag="ones_col")
        nc.gpsimd.memset(ones_col[:], 1.0)

        # per (head, batch) q tiles and per-batch k/v tiles so attention on
        # batch 0 can start while batch 1 is still projecting.
        # v is stored transposed in 128-token column blocks:
        # v_sb[b][:, c*128:(c+1)*128] = v[tokens c*128...][:, d]
        q_sb = [[qkv_pool.tile([128, S], BF16, tag=f"q{h}{b}", name=f"q{h}{b}")
                 for b in range(B)] for h in range(HPC)]
        k_sb = [qkv_pool.tile([128, S], BF16, tag=f"k{b}", name=f"k{b}")
                for b in range(B)]
        v_sb = [qkv_pool.tile([128, S], BF16, tag=f"v{b}", name=f"v{b}")
                for b in range(B)]

        w_pool = ctx.enter_context(tc.tile_pool(name="w", bufs=8))
        w_sb = [w_pool.tile([128, 1024], BF16, tag="wqkv", name=f"w{e}")
                for e in range(8)]

        for _rep in range(repeats):
            # ---- phase 1: qkv projection + rope + v transpose ----
            phase1 = ExitStack()
            x_pool = phase1.enter_context(tc.tile_pool(name="x", bufs=10))
            proj_psum = phase1.enter_context(tc.tile_pool(name="pproj", bufs=4, space="PSUM"))
            tr_psum = phase1.enter_context(tc.tile_pool(name="ptr", bufs=2, space="PSUM"))
            rope_tmp = phase1.enter_context(tc.tile_pool(name="ropetmp", bufs=8))
            v_tmp = phase1.enter_context(tc.tile_pool(name="vtmp", bufs=2))

            QT = 1024  # tokens per quarter

            # first-quarter x and w loads go first so the first matmuls
            # aren't stuck behind the (cold) table/mask loads
            xq0 = []
            for e2 in range(8):
                xt = x_pool.tile([128, 2 * QT], BF16, tag="x", name=f"x0_{e2}")
                nc.sync.dma_start(as3(xt[:]), pair(xT, e2 * 256, 0, QT))
                xq0.append(xt)
                if _rep == 0:
                    nc.scalar.dma_start(as3(w_sb[e2][:]),
                                        pair(wqkvT, e2 * 256, 0, 512))
            if _rep == 0:
                nc.scalar.dma_start(cos_sb[:], cosT[:, :])
                nc.scalar.dma_start(sin_sb[:], sinT[:, :])
                nc.scalar.dma_start(mask_sb[:], masks[:, :])

            for tq in range(T // QT):
                b = tq // 2            # batch this quarter belongs to
                if tq == 0:
                    xq = xq0
                else:
                    xq = []
                    for e2 in range(8):
                        xt = x_pool.tile([128, 2 * QT], BF16, tag="x")
                        nc.sync.dma_start(
                            as3(xt[:]), pair(xT, e2 * 256, tq * QT, (tq + 1) * QT))
                        xq.append(xt)
                # v projected directly in [token, dim] orientation (x tile is
                # the stationary operand) so no PE transpose is needed
                for tb in range(QT // 128):
                    pv = tr_psum.tile([128, 128], FP32, tag="tr")
                    for e in range(NE):
                        nc.tensor.matmul(
                            pv[:],
                            lhsT=xq[e // 2][:, (e % 2) * QT + tb * 128:
                                            (e % 2) * QT + (tb + 1) * 128],
                            rhs=w_sb[e // 2][:, (e % 2) * 512 + 3 * 128:
                                             (e % 2) * 512 + 4 * 128],
                            start=(e == 0), stop=(e == NE - 1),
                        )
                    cblk = tq * QT + tb * 128 - b * S
                    nc.vector.tensor_copy(v_sb[b][:, cblk:cblk + 128], pv[:])
                for dt in range(3):  # q-head0, q-head1, k
                    for half in range(2):
                        ps = proj_psum.tile([128, 512], FP32, tag="proj")
                        for e in range(NE):
                            nc.tensor.matmul(
                                ps[:],
                                lhsT=w_sb[e // 2][:, (e % 2) * 512 + dt * 128:
                                                  (e % 2) * 512 + (dt + 1) * 128],
                                rhs=xq[e // 2][:, (e % 2) * QT + half * 512:
                                               (e % 2) * QT + (half + 1) * 512],
                                start=(e == 0),
                                stop=(e == NE - 1),
                            )
                        gcol = tq * QT + half * 512   # global token offset
                        col = gcol - b * S            # within-batch offset
                        # evacuate psum once (cast to bf16); DVE rope work
                        # stays in fast bf16-SBUF mode
                        vt = v_tmp.tile([128, 512], BF16, tag="vt")
                        nc.scalar.copy(vt[:], ps[:])
                        if True:
                            # rope: rows 0:64 = even head dims, 64:128 = odd
                            dst = q_sb[dt][b] if dt < 2 else k_sb[b]
                            c_top = cos_sb[0:64, gcol:gcol + 512]
                            c_bot = cos_sb[64:128, gcol:gcol + 512]
                            s_top = sin_sb[0:64, gcol:gcol + 512]
                            s_bot = sin_sb[64:128, gcol:gcol + 512]
                            t1 = rope_tmp.tile([64, 512], BF16, tag="rt")
                            t2 = rope_tmp.tile([64, 512], BF16, tag="rt")
                            t3 = rope_tmp.tile([64, 512], BF16, tag="rt")
                            t4 = rope_tmp.tile([64, 512], BF16, tag="rt")
                            top, bot = vt[0:64, :], vt[64:128, :]
                            nc.vector.tensor_mul(t1[:], top, c_top)
                            nc.vector.tensor_mul(t2[:], bot, s_bot)
                            nc.vector.tensor_sub(dst[0:64, col:col + 512], t1[:], t2[:])
                            nc.vector.tensor_mul(t3[:], bot, c_bot)
                            nc.vector.tensor_mul(t4[:], top, s_top)
                            nc.vector.tensor_add(dst[64:128, col:col + 512], t3[:], t4[:])

            phase1.close()

            # ---- phase 2: causal attention per (batch, head) ----
            if _rep == 0:
                wo_pool = ctx.enter_context(tc.tile_pool(name="wo", bufs=16))
            phase2 = ExitStack()
            s_psum = phase2.enter_context(tc.tile_pool(name="ps", bufs=4, space="PSUM"))
            y_psum = phase2.enter_context(tc.tile_pool(name="py", bufs=2, space="PSUM"))
            d_psum = phase2.enter_context(tc.tile_pool(name="pd", bufs=2, space="PSUM"))
            e_pool = phase2.enter_context(tc.tile_pool(name="e", bufs=10))
            d_pool = phase2.enter_context(tc.tile_pool(name="dq", bufs=6))
            sm_pool = phase2.enter_context(tc.tile_pool(name="sm", bufs=2))

            # prefetch o_proj weights under the whole attention phase
            wo_t = []
            for g2 in range(2):
                for yd2 in range(8):
                    wt = wo_pool.tile([128, 2048], BF16, tag="wo",
                                      name=f"wo{g2}_{yd2}")
                    nc.scalar.dma_start(
                        as3(wt[:]),
                        pair(woT, yd2 * 256, g2 * 1024, (g2 + 1) * 1024))
                    wo_t.append(wt)

            for b in range(B):
                for h in range(HPC):
                    for qc in range(S // 512):
                        q0 = qc * 512
                        nkt = (q0 + 512) // 128
                        ps_y = y_psum.tile([128, 512], FP32, tag="y")
                        ps_d = d_psum.tile([1, 512], FP32, tag="d")
                        for g in range(nkt // 4):
                            # columns [0, w0) of a diagonal tile are fully
                            # masked -> skip them in scores/exp/mask/AV
                            quad = []   # (e_t, w0)
                            for i4 in range(4):
                                kt = g * 4 + i4
                                r = kt * 128 - q0
                                w0 = max(r, 0)
                                ps_s = s_psum.tile([128, 512], FP32, tag="s")
                                nc.tensor.matmul(
                                    ps_s[:, w0:512],
                                    lhsT=k_sb[b][:, kt * 128:(kt + 1) * 128],
                                    rhs=q_sb[h][b][:, q0 + w0:q0 + 512],
                                    start=True, stop=True,
                                )
                                e_t = e_pool.tile([128, 512], BF16, tag="e")
                                nc.scalar.activation(
                                    e_t[:, w0:512], ps_s[:, w0:512],
                                    AF.Exp, scale=SCALE)
                                if r >= 0:
                                    off = 384 - r
                                    nc.vector.tensor_mul(
                                        e_t[:, w0:512], e_t[:, w0:512],
                                        mask_sb[:, off + w0:off + 512])
                                quad.append((e_t, w0))
                            for i4, (e_t, w0) in enumerate(quad):
                                kt = g * 4 + i4
                                nc.tensor.matmul(
                                    ps_y[:, w0:512],
                                    lhsT=v_sb[b][:, kt * 128:(kt + 1) * 128],
                                    rhs=e_t[:, w0:512],
                                    start=(kt == 0), stop=(kt == nkt - 1),
                                    skip_group_check=True,
                                )
                            # softmax denominator: sum 4 e-tiles on DVE, then
                            # one PE matmul per quad (4x fewer than per-kt)
                            if quad[3][1] == 0:
                                p0 = d_pool.tile([128, 512], BF16, tag="p")
                                p1 = d_pool.tile([128, 512], BF16, tag="p")
                                q_t = d_pool.tile([128, 512], BF16, tag="p")
                                nc.vector.tensor_add(
                                    p0[:], quad[0][0][:], quad[1][0][:])
                                nc.vector.tensor_add(
                                    p1[:], quad[2][0][:], quad[3][0][:])
                                nc.vector.tensor_add(q_t[:], p0[:], p1[:])
                            else:
                                # diagonal quad: chained partial-width sums
                                q_t = d_pool.tile([128, 512], BF16, tag="p")
                                nc.vector.tensor_copy(q_t[:], quad[0][0][:])
                                for e_t, w0 in quad[1:]:
                                    nc.vector.tensor_add(
                                        q_t[:, w0:512], q_t[:, w0:512],
                                        e_t[:, w0:512])
                            nc.tensor.matmul(
                                ps_d[:], lhsT=ones_col[:], rhs=q_t[:],
                                start=(g == 0), stop=(g == nkt // 4 - 1),
                            )
                        # reciprocal -> partition broadcast -> scale
                        recip = sm_pool.tile([1, 512], FP32, tag="recip")
                        nc.vector.reciprocal(recip[:], ps_d[:])
                        rb = sm_pool.tile([128, 512], FP32, tag="rb")
                        nc.gpsimd.partition_broadcast(rb[:], recip[:])
                        y_t = sm_pool.tile([128, 512], BF16, tag="yt")
                        nc.vector.tensor_mul(y_t[:], ps_y[:], rb[:])
                        # route the two 256-token halves to their dest cores
                        for hf in range(2):
                            j = 2 * qc + hf
                            row = j * HPC * D + h * 128
                            nc.sync.dma_start(
                                a2a_in[b][row:row + 128, :],
                                y_t[:, hf * HSL:(hf + 1) * HSL])
                # ---- per-batch all-to-all, overlapped with remaining work ----
                if single:
                    nc.sync.dma_start(a2a_out[b][:, :], a2a_in[b][:, :])
                else:
                    nc.gpsimd.collective_compute(
                        "AllToAll",
                        mybir.AluOpType.bypass,
                        replica_groups=[list(range(NCORES))],
                        ins=[a2a_in[b][:, :]],
                        outs=[a2a_out[b][:, :]],
                    )

            phase2.close()

            # ---- phase 3: o_proj, one 256-token half-slice per batch ----
            phase3 = ExitStack()
            yag_pool = phase3.enter_context(tc.tile_pool(name="yag", bufs=16))
            o_psum = phase3.enter_context(tc.tile_pool(name="po", bufs=4, space="PSUM"))
            o_pool = phase3.enter_context(tc.tile_pool(name="osb", bufs=4))

            for b in range(B):
                yag = []
                for yd2 in range(8):
                    yt = yag_pool.tile([128, 2 * HSL], BF16, tag="yag")
                    nc.sync.dma_start(
                        as3(yt[:]), pair(a2a_out[b], yd2 * 256, 0, HSL))
                    yag.append(yt)
                for ot in range(NE):
                    ps_o = o_psum.tile([128, HSL], FP32, tag="o")
                    for yd in range(NE):
                        wt = wo_t[(ot // 8) * 8 + yd // 2]
                        oi = ot % 8
                        nc.tensor.matmul(
                            ps_o[:],
                            lhsT=wt[:, (yd % 2) * 1024 + oi * 128:
                                    (yd % 2) * 1024 + (oi + 1) * 128],
                            rhs=yag[yd // 2][:, (yd % 2) * HSL:(yd % 2 + 1) * HSL],
                            start=(yd == 0), stop=(yd == NE - 1))
                    o_sb = o_pool.tile([128, HSL], FP32, tag="osb")
                    nc.vector.tensor_copy(o_sb[:], ps_o[:])
                    nc.sync.dma_start(
                        outT[ot * 128:(ot + 1) * 128, b * HSL:(b + 1) * HSL],
                        o_sb[:])
            phase3.close()

    nc.compile()
    return nc


def _prep_inputs(x, Wq, Wk, Wv, Wo, position_ids):
    bf16 = ml_dtypes.bfloat16
    xT = np.ascontiguousarray(
        x.reshape(T, E).T).astype(bf16)

    # rope permutation: even head-dims first, then odd
    perm = np.concatenate([np.arange(0, D, 2), np.arange(1, D, 2)])
    Wq_p = Wq.reshape(H, D, E)[:, perm, :]
    Wk_p = Wk.reshape(HKV, D, E)[:, perm, :]
    Wv_r = Wv.reshape(HKV, D, E)

    pos = position_ids.astype(np.float64)
    inv_freq = 1.0 / (ROPE_THETA ** (np.arange(0, D, 2, dtype=np.float64) / D))
    freqs = pos[:, None] * inv_freq[None, :]            # [S, 64]
    cosT = np.tile(np.cos(freqs).T, (2, B)).astype(bf16)  # [128, T] (dup halves)
    sinT = np.tile(np.sin(freqs).T, (2, B)).astype(bf16)

    # causal band mask: masks[k, j] = 1 if k + 384 <= j; the r-th diagonal
    # block mask [k, q] = (k + 128r <= q) is the slice [:, 384-128r:896-128r]
    kk = np.arange(128)[:, None]
    jj = np.arange(896)[None, :]
    masks = np.ascontiguousarray(
        (kk + 384 <= jj).astype(np.float32)).astype(bf16)

    woT = np.ascontiguousarray(Wo.T).astype(bf16)       # [yd, o]

    in_maps = []
    for c in range(NCORES):
        wq_c = Wq_p[2 * c:2 * c + 2].reshape(HPC * D, E)     # [256, E]
        g = c // 2
        wqkvT = np.concatenate(
            [wq_c.T, Wk_p[g].T, Wv_r[g].T], axis=1).astype(bf16)  # [E, 512]
        in_maps.append({
            "xT": xT,
            "wqkvT": np.ascontiguousarray(wqkvT),
            "woT": woT,
            "cosT": cosT,
            "sinT": sinT,
            "masks": masks,
        })
    return in_maps


def kernel(x, Wq, Wk, Wv, Wo, position_ids):
    global _cached_nc
    if _cached_nc is None:
        _cached_nc = _build_nc()
    nc = _cached_nc

    in_maps = _prep_inputs(
        np.asarray(x, np.float32), np.asarray(Wq, np.float32),
        np.asarray(Wk, np.float32), np.asarray(Wv, np.float32),
        np.asarray(Wo, np.float32), np.asarray(position_ids))

    res = run_bass_kernel_spmd(nc, in_maps, core_ids=list(range(NCORES)))

    # core c's outT is [E, 512]: cols 0:256 = batch-0 tokens [c*256,(c+1)*256),
    # cols 256:512 = batch-1 same token range
    HSL = TSL // 2
    out = np.empty((B, S, E), np.float32)
    for c in range(NCORES):
        o = res.results[c]["outT"]
        out[0, c * HSL:(c + 1) * HSL, :] = o[:, 0:HSL].T
        out[1, c * HSL:(c + 1) * HSL, :] = o[:, HSL:2 * HSL].T
    return np.ascontiguousarray(out)



# revision 13
# speedup vs baseline: 1.3451x; 1.2482x over previous
"""Causal GQA self-attention with RoPE on 8 Trainium2 NeuronCores.

Sharding: tensor-parallel over heads. Each core owns 2 q-heads and their
(shared) kv-head: it projects q/k/v for all 4096 tokens, applies RoPE, runs
causal attention, then an AllToAll redistributes attention outputs so each
core o-projects a 512-token slice with the full Wo. Host assembles slices.

All matmuls run in bf16 with fp32 PSUM accumulation. The RoPE interleaved
pair rotation is turned into a contiguous half rotation by permuting the
rows of Wq/Wk on the host (even head-dims first); q.k dot products are
invariant under applying the same permutation to q and k.

Shapes (hardcoded from the problem spec):
  x [2, 2048, 2048] f32, Wq [2048, 2048], Wk/Wv [512, 2048], Wo [2048, 2048],
  position_ids [2048] i32.  16 q-heads, 4 kv-heads, head_dim 128.
"""

from contextlib import ExitStack

import ml_dtypes
import numpy as np

import concourse.bass as bass
import concourse.tile as tile
from concourse import bacc, mybir
from concourse.bass_utils import run_bass_kernel_spmd
from concourse.masks import make_identity

B, S, E = 2, 2048, 2048
H, HKV, D = 16, 4, 128
NCORES = 8
HPC = H // NCORES          # q-heads per core
T = B * S                  # 4096 flattened tokens
TSL = T // NCORES          # 512-token o_proj slice per core
NE = E // 128              # 16 contraction chunks
ROPE_THETA = 10000.0
SCALE = 1.0 / float(np.sqrt(D))

BF16 = mybir.dt.bfloat16
FP32 = mybir.dt.float32
AF = mybir.ActivationFunctionType

_cached_nc = None


def _build_nc(single=False, repeats=1):
    """single=True: 1-core variant with the collective replaced by a DMA copy
    (for TimelineSim cost-model analysis only).  repeats>1 unrolls the whole
    body N times inside one NEFF (used to measure steady-state device time
    by slope, since per-dispatch host overhead dominates wall clock)."""
    nc = bacc.Bacc(None, target_bir_lowering=False, debug=False,
                   num_devices=1 if single else NCORES)

    xT = nc.dram_tensor("xT", [E, T], BF16, kind="ExternalInput")
    wqkvT = nc.dram_tensor("wqkvT", [E, 512], BF16, kind="ExternalInput")
    woT = nc.dram_tensor("woT", [E, E], BF16, kind="ExternalInput")
    cosT = nc.dram_tensor("cosT", [128, T], BF16, kind="ExternalInput")
    sinT = nc.dram_tensor("sinT", [128, T], BF16, kind="ExternalInput")
    masks = nc.dram_tensor("masks", [128, 896], BF16, kind="ExternalInput")
    outT = nc.dram_tensor("outT", [E, TSL], FP32, kind="ExternalOutput")

    # per-batch all-to-all buffers: dest core j gets tokens [j*256,(j+1)*256)
    # of that batch, so each collective is half-size and overlaps compute
    HSL = TSL // 2   # 256 tokens per (core, batch) output slice
    a2a_in = [nc.dram_tensor(f"a2a_in{b}", [NCORES * HPC * D, HSL], BF16)
              for b in range(B)]
    a2a_out = [nc.dram_tensor(f"a2a_out{b}", [NCORES * HPC * D, HSL], BF16)
               for b in range(B)]

    def pair(dram, r0, c0, c1):
        # [256 rows, c] dram block -> [128, 2, c] AP (two row-chunks side
        # by side) so two contraction chunks load in one DMA
        return dram[r0:r0 + 256, c0:c1].rearrange("(c p) t -> p c t", c=2)

    def as3(ap, c=2):
        return ap.rearrange("p (c t) -> p c t", c=c)

    with tile.TileContext(nc) as tc, ExitStack() as ctx:
        # ---- persistent SBUF ----
        const_pool = ctx.enter_context(tc.tile_pool(name="const", bufs=1))
        qkv_pool = ctx.enter_context(tc.tile_pool(name="qkv", bufs=1))

        cos_sb = const_pool.tile([128, T], BF16, tag="cos")
        sin_sb = const_pool.tile([128, T], BF16, tag="sin")
        # causal band: mask_sb[k, j] = (k + 384 <= j); the r-th diagonal
        # 128-block mask is the slice [:, 384-128r : 896-128r]
        mask_sb = const_pool.tile([128, 896], BF16, tag="mask")

        ones_col = const_pool.tile([128, 1], BF16, tag="ones_col")
        nc.gpsimd.memset(ones_col[:], 1.0)

        # per (head, batch) q tiles and per-batch k/v tiles so attention on
        # batch 0 can start while batch 1 is still projecting.
        # v is stored transposed in 128-token column blocks:
        # v_sb[b][:, c*128:(c+1)*128] = v[tokens c*128...][:, d]
        q_sb = [[qkv_pool.tile([128, S], BF16, tag=f"q{h}{b}", name=f"q{h}{b}")
                 for b in range(B)] for h in range(HPC)]
        k_sb = [qkv_pool.tile([128, S], BF16, tag=f"k{b}", name=f"k{b}")
                for b in range(B)]
        v_sb = [qkv_pool.tile([128, S], BF16, tag=f"v{b}", name=f"v{b}")
                for b in range(B)]

        w_pool = ctx.enter_context(tc.tile_pool(name="w", bufs=8))
        w_sb = [w_pool.tile([128, 1024], BF16, tag="wqkv", name=f"w{e}")
                for e in range(8)]

        for _rep in range(repeats):
            # ---- phase 1: qkv projection + rope + v transpose ----
            phase1 = ExitStack()
            x_pool = phase1.enter_context(tc.tile_pool(name="x", bufs=10))
            proj_psum = phase1.enter_context(tc.tile_pool(name="pproj", bufs=4, space="PSUM"))
            tr_psum = phase1.enter_context(tc.tile_pool(name="ptr", bufs=2, space="PSUM"))
            rope_tmp = phase1.enter_context(tc.tile_pool(name="ropetmp", bufs=8))
            v_tmp = phase1.enter_context(tc.tile_pool(name="vtmp", bufs=2))

            QT = 1024  # tokens per quarter

            # first-quarter x and w loads go first so the first matmuls
            # aren't stuck behind the (cold) table/mask loads
            xq0 = []
            for e2 in range(8):
                xt = x_pool.tile([128, 2 * QT], BF16, tag="x", name=f"x0_{e2}")
                nc.sync.dma_start(as3(xt[:]), pair(xT, e2 * 256, 0, QT))
                xq0.append(xt)
                if _rep == 0:
                    nc.scalar.dma_start(as3(w_sb[e2][:]),
                                        pair(wqkvT, e2 * 256, 0, 512))
            if _rep == 0:
                nc.scalar.dma_start(cos_sb[:], cosT[:, :])
                nc.scalar.dma_start(sin_sb[:], sinT[:, :])
                nc.scalar.dma_start(mask_sb[:], masks[:, :])

            for tq in range(T // QT):
                b = tq // 2            # batch this quarter belongs to
                if tq == 0:
                    xq = xq0
                else:
                    xq = []
                    for e2 in range(8):
                        xt = x_pool.tile([128, 2 * QT], BF16, tag="x")
                        nc.sync.dma_start(
                            as3(xt[:]), pair(xT, e2 * 256, tq * QT, (tq + 1) * QT))
                        xq.append(xt)
                # v projected directly in [token, dim] orientation (x tile is
                # the stationary operand) so no PE transpose is needed
                for tb in range(QT // 128):
                    pv = tr_psum.tile([128, 128], FP32, tag="tr")
                    for e in range(NE):
                        nc.tensor.matmul(
                            pv[:],
                            lhsT=xq[e // 2][:, (e % 2) * QT + tb * 128:
                                            (e % 2) * QT + (tb + 1) * 128],
                            rhs=w_sb[e // 2][:, (e % 2) * 512 + 3 * 128:
                                             (e % 2) * 512 + 4 * 128],
                            start=(e == 0), stop=(e == NE - 1),
                        )
                    cblk = tq * QT + tb * 128 - b * S
                    nc.vector.tensor_copy(v_sb[b][:, cblk:cblk + 128], pv[:])
                for dt in range(3):  # q-head0, q-head1, k
                    for half in range(2):
                        ps = proj_psum.tile([128, 512], FP32, tag="proj")
                        for e in range(NE):
                            nc.tensor.matmul(
                                ps[:],
                                lhsT=w_sb[e // 2][:, (e % 2) * 512 + dt * 128:
                                                  (e % 2) * 512 + (dt + 1) * 128],
                                rhs=xq[e // 2][:, (e % 2) * QT + half * 512:
                                               (e % 2) * QT + (half + 1) * 512],
                                start=(e == 0),
                                stop=(e == NE - 1),
                            )
                        gcol = tq * QT + half * 512   # global token offset
                        col = gcol - b * S            # within-batch offset
                        # evacuate psum once (cast to bf16); DVE rope work
                        # stays in fast bf16-SBUF mode
                        vt = v_tmp.tile([128, 512], BF16, tag="vt")
                        nc.scalar.copy(vt[:], ps[:])
                        if True:
                            # rope: rows 0:64 = even head dims, 64:128 = odd
                            dst = q_sb[dt][b] if dt < 2 else k_sb[b]
                            c_top = cos_sb[0:64, gcol:gcol + 512]
                            c_bot = cos_sb[64:128, gcol:gcol + 512]
                            s_top = sin_sb[0:64, gcol:gcol + 512]
                            s_bot = sin_sb[64:128, gcol:gcol + 512]
                            t1 = rope_tmp.tile([64, 512], BF16, tag="rt")
                            t2 = rope_tmp.tile([64, 512], BF16, tag="rt")
                            t3 = rope_tmp.tile([64, 512], BF16, tag="rt")
                            t4 = rope_tmp.tile([64, 512], BF16, tag="rt")
                            top, bot = vt[0:64, :], vt[64:128, :]
                            nc.vector.tensor_mul(t1[:], top, c_top)
                            nc.vector.tensor_mul(t2[:], bot, s_bot)
                            nc.vector.tensor_sub(dst[0:64, col:col + 512], t1[:], t2[:])
                            nc.vector.tensor_mul(t3[:], bot, c_bot)
                            nc.vector.tensor_mul(t4[:], top, s_top)
                            nc.vector.tensor_add(dst[64:128, col:col + 512], t3[:], t4[:])

            phase1.close()

            # ---- phase 2: causal attention per (batch, head) ----
            if _rep == 0:
                wo_pool = ctx.enter_context(tc.tile_pool(name="wo", bufs=16))
            phase2 = ExitStack()
            s_psum = phase2.enter_context(tc.tile_pool(name="ps", bufs=4, space="PSUM"))
            y_psum = phase2.enter_context(tc.tile_pool(name="py", bufs=2, space="PSUM"))
            d_psum = phase2.enter_context(tc.tile_pool(name="pd", bufs=2, space="PSUM"))
            e_pool = phase2.enter_context(tc.tile_pool(name="e", bufs=10))
            d_pool = phase2.enter_context(tc.tile_pool(name="dq", bufs=6))
            sm_pool = phase2.enter_context(tc.tile_pool(name="sm", bufs=2))

            # prefetch o_proj weights under the whole attention phase
            wo_t = []
            for g2 in range(2):
                for yd2 in range(8):
                    wt = wo_pool.tile([128, 2048], BF16, tag="wo",
                                      name=f"wo{g2}_{yd2}")
                    nc.scalar.dma_start(
                        as3(wt[:]),
                        pair(woT, yd2 * 256, g2 * 1024, (g2 + 1) * 1024))
                    wo_t.append(wt)

            for b in range(B):
                for h in range(HPC):
                    for qc in range(S // 512):
                        q0 = qc * 512
                        nkt = (q0 + 512) // 128
                        ps_y = y_psum.tile([128, 512], FP32, tag="y")
                        ps_d = d_psum.tile([1, 512], FP32, tag="d")
                        for g in range(nkt // 4):
                            # columns [0, w0) of a diagonal tile are fully
                            # masked -> skip them in scores/exp/mask/AV
                            quad = []   # (e_t, w0)
                            for i4 in range(4):
                                kt = g * 4 + i4
                                r = kt * 128 - q0
                                w0 = max(r, 0)
                                ps_s = s_psum.tile([128, 512], FP32, tag="s")
                                nc.tensor.matmul(
                                    ps_s[:, w0:512],
                                    lhsT=k_sb[b][:, kt * 128:(kt + 1) * 128],
                                    rhs=q_sb[h][b][:, q0 + w0:q0 + 512],
                                    start=True, stop=True,
                                )
                                e_t = e_pool.tile([128, 512], BF16, tag="e")
                                nc.scalar.activation(
                                    e_t[:, w0:512], ps_s[:, w0:512],
                                    AF.Exp, scale=SCALE)
                                if r >= 0:
                                    off = 384 - r
                                    nc.vector.tensor_mul(
                                        e_t[:, w0:512], e_t[:, w0:512],
                                        mask_sb[:, off + w0:off + 512])
                                quad.append((e_t, w0))
                            for i4, (e_t, w0) in enumerate(quad):
                                kt = g * 4 + i4
                                nc.tensor.matmul(
                                    ps_y[:, w0:512],
                                    lhsT=v_sb[b][:, kt * 128:(kt + 1) * 128],
                                    rhs=e_t[:, w0:512],
                                    start=(kt == 0), stop=(kt == nkt - 1),
                                    skip_group_check=True,
                                )
                            # softmax denominator: sum 4 e-tiles on DVE, then
                            # one PE matmul per quad (4x fewer than per-kt)
                            if quad[3][1] == 0:
                                p0 = d_pool.tile([128, 512], BF16, tag="p")
                                p1 = d_pool.tile([128, 512], BF16, tag="p")
                                q_t = d_pool.tile([128, 512], BF16, tag="p")
                                nc.vector.tensor_add(
                                    p0[:], quad[0][0][:], quad[1][0][:])
                                nc.vector.tensor_add(
                                    p1[:], quad[2][0][:], quad[3][0][:])
                                nc.vector.tensor_add(q_t[:], p0[:], p1[:])
                            else:
                                # diagonal quad: chained partial-width sums
                                q_t = d_pool.tile([128, 512], BF16, tag="p")
                                nc.vector.tensor_copy(q_t[:], quad[0][0][:])
                                for e_t, w0 in quad[1:]:
                                    nc.vector.tensor_add(
                                        q_t[:, w0:512], q_t[:, w0:512],
                                        e_t[:, w0:512])
                            nc.tensor.matmul(
                                ps_d[:], lhsT=ones_col[:], rhs=q_t[:],
                                start=(g == 0), stop=(g == nkt // 4 - 1),
                            )
                        # reciprocal -> partition broadcast -> scale
                        recip = sm_pool.tile([1, 512], FP32, tag="recip")
                        nc.vector.reciprocal(recip[:], ps_d[:])
                        rb = sm_pool.tile([128, 512], FP32, tag="rb")
                        nc.gpsimd.partition_broadcast(rb[:], recip[:])
                        y_t = sm_pool.tile([128, 512], BF16, tag="yt")
                        nc.vector.tensor_mul(y_t[:], ps_y[:], rb[:])
                        # route the two 256-token halves to their dest cores
                        for hf in range(2):
                            j = 2 * qc + hf
                            row = j * HPC * D + h * 128
                            nc.sync.dma_start(
                                a2a_in[b][row:row + 128, :],
                                y_t[:, hf * HSL:(hf + 1) * HSL])
                # ---- per-batch all-to-all, overlapped with remaining work ----
                if single:
                    nc.sync.dma_start(a2a_out[b][:, :], a2a_in[b][:, :])
                else:
                    nc.gpsimd.collective_compute(
                        "AllToAll",
                        mybir.AluOpType.bypass,
                        replica_groups=[list(range(NCORES))],
                        ins=[a2a_in[b][:, :]],
                        outs=[a2a_out[b][:, :]],
                    )

            phase2.close()

            # ---- phase 3: o_proj, one 256-token half-slice per batch ----
            phase3 = ExitStack()
            yag_pool = phase3.enter_context(tc.tile_pool(name="yag", bufs=16))
            o_psum = phase3.enter_context(tc.tile_pool(name="po", bufs=4, space="PSUM"))
            o_pool = phase3.enter_context(tc.tile_pool(name="osb", bufs=4))

            for b in range(B):
                yag = []
                for yd2 in range(8):
                    yt = yag_pool.tile([128, 2 * HSL], BF16, tag="yag")
                    nc.sync.dma_start(
                        as3(yt[:]), pair(a2a_out[b], yd2 * 256, 0, HSL))
                    yag.append(yt)
                for ot in range(NE):
                    ps_o = o_psum.tile([128, HSL], FP32, tag="o")
                    for yd in range(NE):
                        wt = wo_t[(ot // 8) * 8 + yd // 2]
                        oi = ot % 8
                        nc.tensor.matmul(
                            ps_o[:],
                            lhsT=wt[:, (yd % 2) * 1024 + oi * 128:
                                    (yd % 2) * 1024 + (oi + 1) * 128],
                            rhs=yag[yd // 2][:, (yd % 2) * HSL:(yd % 2 + 1) * HSL],
                            start=(yd == 0), stop=(yd == NE - 1))
                    o_sb = o_pool.tile([128, HSL], FP32, tag="osb")
                    nc.vector.tensor_copy(o_sb[:], ps_o[:])
                    nc.sync.dma_start(
                        outT[ot * 128:(ot + 1) * 128, b * HSL:(b + 1) * HSL],
                        o_sb[:])
            phase3.close()

    nc.compile()
    return nc


def _prep_inputs(x, Wq, Wk, Wv, Wo, position_ids):
    bf16 = ml_dtypes.bfloat16
    xT = np.ascontiguousarray(
        x.reshape(T, E).T).astype(bf16)

    # rope permutation: even head-dims first, then odd
    perm = np.concatenate([np.arange(0, D, 2), np.arange(1, D, 2)])
    Wq_p = Wq.reshape(H, D, E)[:, perm, :]
    Wk_p = Wk.reshape(HKV, D, E)[:, perm, :]
    Wv_r = Wv.reshape(HKV, D, E)

    pos = position_ids.astype(np.float64)
    inv_freq = 1.0 / (ROPE_THETA ** (np.arange(0, D, 2, dtype=np.float64) / D))
    freqs = pos[:, None] * inv_freq[None, :]            # [S, 64]
    cosT = np.tile(np.cos(freqs).T, (2, B)).astype(bf16)  # [128, T] (dup halves)
    sinT = np.tile(np.sin(freqs).T, (2, B)).astype(bf16)

    # causal band mask: masks[k, j] = 1 if k + 384 <= j; the r-th diagonal
    # block mask [k, q] = (k + 128r <= q) is the slice [:, 384-128r:896-128r]
    kk = np.arange(128)[:, None]
    jj = np.arange(896)[None, :]
    masks = np.ascontiguousarray(
        (kk + 384 <= jj).astype(np.float32)).astype(bf16)

    woT = np.ascontiguousarray(Wo.T).astype(bf16)       # [yd, o]

    in_maps = []
    for c in range(NCORES):
        wq_c = Wq_p[2 * c:2 * c + 2].reshape(HPC * D, E)     # [256, E]
        g = c // 2
        wqkvT = np.concatenate(
            [wq_c.T, Wk_p[g].T, Wv_r[g].T], axis=1).astype(bf16)  # [E, 512]
        in_maps.append({
            "xT": xT,
            "wqkvT": np.ascontiguousarray(wqkvT),
            "woT": woT,
            "cosT": cosT,
            "sinT": sinT,
            "masks": masks,
        })
    return in_maps


def kernel(x, Wq, Wk, Wv, Wo, position_ids):
    global _cached_nc
    if _cached_nc is None:
        _cached_nc = _build_nc()
    nc = _cached_nc

    in_maps = _prep_inputs(
        np.asarray(x, np.float32), np.asarray(Wq, np.float32),
        np.asarray(Wk, np.float32), np.asarray(Wv, np.float32),
        np.asarray(Wo, np.float32), np.asarray(position_ids))

    res = run_bass_kernel_spmd(nc, in_maps, core_ids=list(range(NCORES)))

    # core c's outT is [E, 512]: cols 0:256 = batch-0 tokens [c*256,(c+1)*256),
    # cols 256:512 = batch-1 same token range
    HSL = TSL // 2
    out = np.empty((B, S, E), np.float32)
    for c in range(NCORES):
        o = res.results[c]["outT"]
        out[0, c * HSL:(c + 1) * HSL, :] = o[:, 0:HSL].T
        out[1, c * HSL:(c + 1) * HSL, :] = o[:, HSL:2 * HSL].T
    return np.ascontiguousarray(out)

